# revision 17
# baseline (speedup 1.0000x reference)
"""Trainium2 Bass kernel for 2-layer GAT (nn_GAT_59133109732231).

Self-contained: kernel(**inputs) -> np.ndarray [100000, 2] float32.

Distribution (8 NeuronCores, SPMD):
  - nodes permuted so core c owns S_SC=120 superchunks x 128 output rows
    (row 127 of each superchunk = trash row for pad edges).
  - a superchunk owns <=127 dst nodes and all their in-edges (self-loops
    handled separately, locally), packed into 4 segments of SEG=256 edge
    slots keyed by src quadrant (= src owner core pair).
  - per layer: transform local nodes with augmented weights
    [W | W@a_src | W@a_dst]; record = [h0+b0 | h1+b1 | asrc | adst] in
    bf16 at 768B row stride; ONE 8-rank AllGather -> full record table;
    per 4-superchunk block: dma_gather records by src row (int16), a
    second local dma_gather brings the dst node's adst per edge slot;
    edge weights w = exp(prelu(asrc+adst)) scale the gathered h via DVE
    broadcast muls; one-hot matmuls scatter-accumulate numerators and
    denominators in PSUM.  All Act-engine functions stay inside the
    exp_and_others table set (prelu/exp/copy/relu/tanh) so the act
    function table is loaded once (sigmoid is computed via tanh).
"""
import os
import sys

import numpy as np
import ml_dtypes

for _p in ("/opt/trn_rl_repo", "/root/.axon_site/_ro/trn_rl_repo"):
    if os.path.isdir(_p) and _p not in sys.path:
        sys.path.append(_p)

N = 100000
NCORES = 8
S_SC = 120
SEG = 256
ROWS_CORE = S_SC * 128          # 15360
QROWS = 2 * ROWS_CORE           # 30720 rows per quadrant table
GE = 4                          # superchunks per gather block
NBLK = S_SC // GE               # 30
NEG_SLOPE = 0.2
REC = 384                       # record row stride (bf16 cols) = 768 B
PAY = 260                       # payload cols: h(256) + asrc(2) + adst(2)

bf16 = ml_dtypes.bfloat16


# ----------------------------------------------------------------- host prep
def build_plan(edge_index):
    edge_index = np.asarray(edge_index)
    src = edge_index[0].astype(np.int64)
    dst = edge_index[1].astype(np.int64)

    deg = np.bincount(dst, minlength=N)
    order = np.argsort(-deg, kind="stable")
    owner = np.empty(N, dtype=np.int32)
    snake = np.tile(np.concatenate([np.arange(8), np.arange(7, -1, -1)]), N // 16 + 1)[:N]
    owner[order] = snake.astype(np.int32)

    e_q = (owner[src] // 2).astype(np.int32)
    qd = np.zeros((N, 4), dtype=np.int32)
    np.add.at(qd, (dst, e_q), 1)

    sc_of = np.empty(N, dtype=np.int32)
    row_of = np.empty(N, dtype=np.int32)
    for c in range(8):
        nodes = np.where(owner == c)[0]
        nodes = nodes[np.argsort(-deg[nodes], kind="stable")]
        loads = np.zeros((S_SC, 4), dtype=np.int32)
        counts = np.zeros(S_SC, dtype=np.int32)
        tot = np.zeros(S_SC, dtype=np.int32)
        big = 1.0e9
        for n in nodes:
            after = loads + qd[n][None, :]
            ok = (after <= SEG).all(axis=1) & (counts < 127)
            key = after.max(axis=1).astype(np.float64) + tot * 1e-6 + (~ok) * big
            k = int(np.argmin(key))
            assert ok[k], "packing failed"
            sc_of[n] = k
            row_of[n] = counts[k]
            counts[k] += 1
            loads[k] += qd[n]
            tot[k] += deg[n]
    rowq_of = ((owner % 2) * ROWS_CORE + sc_of * 128 + row_of).astype(np.int32)

    e_core = owner[dst]
    e_sc = sc_of[dst]
    e_rowq = rowq_of[src]
    e_dloc = row_of[dst]

    plans = []
    for c in range(8):
        eidx = np.zeros((S_SC, 4, SEG), dtype=np.int16)
        dloc = np.full((S_SC, 4, SEG), 127, dtype=np.int32)
        m = e_core == c
        sc_c, q_c, rq_c, dl_c = e_sc[m], e_q[m], e_rowq[m], e_dloc[m]
        o = np.lexsort((q_c, sc_c))
        sc_c, q_c, rq_c, dl_c = sc_c[o], q_c[o], rq_c[o], dl_c[o]
        key = sc_c * 4 + q_c
        pos = np.arange(len(key)) - np.searchsorted(key, key, side="left")
        assert pos.max() < SEG
        eidx[sc_c, q_c, pos] = rq_c.astype(np.int16)
        dloc[sc_c, q_c, pos] = dl_c
        plans.append(dict(eidx=eidx, dloc=dloc))
    return dict(owner=owner, sc_of=sc_of, row_of=row_of, plans=plans)


def _wrap16(idxs):
    """SBUF index layout for dma_gather: [16, n//16] wrapped, tiled to 128."""
    n = idxs.shape[0]
    w = idxs.reshape(n // 16, 16).T
    return np.tile(w, (8, 1))


def make_core_inputs(plan, inputs):
    x = np.asarray(inputs["x"], dtype=np.float32)

    def amat(a):
        a = np.asarray(a, dtype=np.float32)
        m = np.zeros((256, 2), dtype=np.float32)
        m[0:128, 0] = a[0]
        m[128:256, 1] = a[1]
        return m

    W1 = np.asarray(inputs["W1"], dtype=np.float32)
    W2 = np.asarray(inputs["W2"], dtype=np.float32)
    W1aug = np.concatenate(
        [W1, W1 @ amat(inputs["a_src1"]), W1 @ amat(inputs["a_dst1"])], axis=1)
    W2aug = np.concatenate(
        [W2, W2 @ amat(inputs["a_src2"]), W2 @ amat(inputs["a_dst2"])], axis=1)

    def bext(b):
        e = np.zeros((128, PAY), dtype=np.float32)
        e[:, 0:256] = np.asarray(b, dtype=np.float32)[None, :]
        return e

    shared = dict(
        w1aug=W1aug.astype(bf16),
        w2aug=W2aug.astype(bf16),
        wp1=np.asarray(inputs["Wp1"], dtype=np.float32).astype(bf16),
        wp2=np.asarray(inputs["Wp2"], dtype=np.float32).astype(bf16),
        b1ext=bext(inputs["b1"]), b2ext=bext(inputs["b2"]),
        bp1col=np.asarray(inputs["bp1"], dtype=np.float32).reshape(128, 1).copy(),
        bp2col=np.concatenate(
            [0.5 * np.asarray(inputs["bp2"], dtype=np.float32).reshape(2, 1),
             np.zeros((126, 1), np.float32)], axis=0),
        ident=np.eye(128, dtype=np.float32).astype(bf16),
        iotam=np.broadcast_to(np.arange(128, dtype=np.float32).astype(bf16)[None, None, :],
                              (128, 8, 128)).copy(),
    )

    owner, sc_of, row_of = plan["owner"], plan["sc_of"], plan["row_of"]
    cores = []
    for c in range(8):
        xp = np.zeros((ROWS_CORE, 128), dtype=np.float32)
        nodes = np.where(owner == c)[0]
        xp[sc_of[nodes] * 128 + row_of[nodes]] = x[nodes]
        xpT = np.ascontiguousarray(xp.T).astype(bf16)  # [128 feat, ROWS_CORE]

        p = plan["plans"][c]
        dl = p["dloc"].reshape(S_SC, 8, 128)  # [sc, chunk j=2q+jj, slot]

        # main gather indices: per (block b, quadrant q): 1024 idxs
        # out col = jl*2 + jj, partition = slot%128
        eidx_w = np.zeros((128, NBLK * 4 * 64), dtype=np.int16)
        for b in range(NBLK):
            for q in range(4):
                idxs = p["eidx"][b * GE:(b + 1) * GE, q, :].reshape(-1)
                col0 = (b * 4 + q) * 64
                eidx_w[:, col0:col0 + 64] = _wrap16(idxs)

        # adst gather indices: per block: 4096 idxs into local shard rows,
        # out col = jl*8 + jj*4 + q  (jj-major for contiguous slices)
        eidx2_w = np.zeros((128, NBLK * 256), dtype=np.int16)
        for b in range(NBLK):
            idxs2 = np.empty(GE * 8 * 128, dtype=np.int16)
            for jl in range(GE):
                k = b * GE + jl
                for jj in range(2):
                    for q in range(4):
                        col = jl * 8 + jj * 4 + q
                        j = 2 * q + jj
                        idxs2[col * 128:(col + 1) * 128] = k * 128 + dl[k, j, :]
            eidx2_w[:, b * 256:(b + 1) * 256] = _wrap16(idxs2)

        dloc_b = np.ascontiguousarray(
            dl.transpose(2, 0, 1).reshape(128, S_SC * 8)).astype(np.float32).astype(bf16)
        cores.append(dict(xpT=xpT, eidx=eidx_w, eidx2=eidx2_w, dstloc=dloc_b))
    return cores, shared


# -------------------------------------------------------------- bass program
def build_nc(skip_ag=False, use_prelu=True):
    import concourse.bass as bass
    import concourse.bacc as bacc
    import concourse.mybir as mybir
    import concourse.tile as tile

    F32, BF, I16 = mybir.dt.float32, mybir.dt.bfloat16, mybir.dt.int16
    AF = mybir.ActivationFunctionType
    ALU = mybir.AluOpType

    nc = bacc.Bacc("TRN2", target_bir_lowering=False, debug=False, num_devices=8)

    din = {}
    for name, shape, dt in [
        ("xpT", [128, ROWS_CORE], BF),
        ("eidx", [128, NBLK * 4 * 64], I16),
        ("eidx2", [128, NBLK * 256], I16),
        ("dstloc", [128, S_SC * 8], BF),
        ("w1aug", [128, PAY], BF),
        ("w2aug", [256, PAY], BF),
        ("wp1", [256, 128], BF),
        ("wp2", [128, 2], BF),
        ("b1ext", [128, PAY], F32), ("b2ext", [128, PAY], F32),
        ("bp1col", [128, 1], F32), ("bp2col", [128, 1], F32),
        ("ident", [128, 128], BF),
        ("iotam", [128, 8, 128], BF),
    ]:
        din[name] = nc.dram_tensor(name, shape, dt, kind="ExternalInput")
    y_d = nc.dram_tensor("y", [2, ROWS_CORE], F32, kind="ExternalOutput")
    shard = [nc.dram_tensor(f"shard{l}", [ROWS_CORE, REC], BF, kind="Internal")
             for l in range(2)]
    table = [nc.dram_tensor(f"table{l}", [8 * ROWS_CORE, REC], BF, kind="Internal",
                            addr_space="Shared") for l in range(2)]

    def lrelu_exp(out_ap, in_ap, tmp_ap):
        """out = exp(leaky_relu(in)) staying inside the exp act table set."""
        if use_prelu:
            nc.scalar.activation(tmp_ap, in_ap, AF.Prelu, alpha=NEG_SLOPE)
            nc.scalar.activation(out_ap, tmp_ap, AF.Exp)
        else:
            # exp(lrelu(z)) == max(exp(z), exp(0.2 z))
            nc.scalar.activation(out_ap, in_ap, AF.Exp)
            nc.scalar.activation(tmp_ap, in_ap, AF.Exp, scale=NEG_SLOPE)
            nc.vector.tensor_tensor(out_ap, out_ap, tmp_ap, ALU.max)

    with tile.TileContext(nc) as tc:
        import contextlib
        ctx = contextlib.ExitStack()
        with ctx:
            pp = ctx.enter_context(tc.tile_pool(name="pp", bufs=1))
            sb = ctx.enter_context(tc.tile_pool(name="sb", bufs=3))
            gp = ctx.enter_context(tc.tile_pool(name="gp", bufs=2))
            ps_po = ctx.enter_context(tc.tile_pool(name="ps_po", bufs=4, space="PSUM"))
            ps_t = ctx.enter_context(tc.tile_pool(name="ps_t", bufs=1, space="PSUM"))

            # persistent SBUF
            P = {}
            for name in ("eidx", "eidx2", "dstloc", "w1aug", "wp2",
                         "b1ext", "b2ext", "bp1col", "bp2col", "ident", "iotam"):
                t = pp.tile(list(din[name].shape), din[name].dtype, tag=f"p_{name}")
                nc.sync.dma_start(t[:], din[name].ap())
                P[name] = t
            w2s = pp.tile([128, 2, PAY], BF, tag="p_w2")
            nc.sync.dma_start(w2s[:, 0, :], din["w2aug"].ap()[0:128, :])
            nc.sync.dma_start(w2s[:, 1, :], din["w2aug"].ap()[128:256, :])
            wp1s = pp.tile([128, 2, 128], BF, tag="p_wp1")
            nc.sync.dma_start(wp1s[:, 0, :], din["wp1"].ap()[0:128, :])
            nc.sync.dma_start(wp1s[:, 1, :], din["wp1"].ap()[128:256, :])
            aa0 = pp.tile([128, S_SC * 4], F32, tag="p_aa0")
            aa1 = pp.tile([128, S_SC * 4], F32, tag="p_aa1")
            aa = [aa0, aa1]

            # ---------------- phase T1: transform x, build layer-1 records
            for b in range(NBLK):
                x4 = sb.tile([128, GE, 128], BF, tag="x4")
                nc.scalar.dma_start(
                    x4[:],
                    din["xpT"].ap()[:, b * GE * 128:(b + 1) * GE * 128]
                    .rearrange("f (g p) -> f g p", g=GE))
                rec4 = sb.tile([128, GE, PAY], BF, tag="rec4")
                for jl in range(GE):
                    k = b * GE + jl
                    ph = ps_t.tile([128, PAY], F32, tag="ph")
                    nc.tensor.matmul(ph[:], lhsT=x4[:, jl, :], rhs=P["w1aug"][:],
                                     start=True, stop=True)
                    nc.vector.tensor_tensor(rec4[:, jl, :], ph[:], P["b1ext"][:],
                                            ALU.add)
                    nc.scalar.activation(aa[0][:, 4 * k:4 * k + 4], ph[:, 256:260],
                                         AF.Copy)
                nc.sync.dma_start(
                    shard[0].ap()[b * GE * 128:(b + 1) * GE * 128, 0:PAY]
                    .rearrange("(g p) c -> p g c", g=GE),
                    rec4[:])

            # ---------------- per-layer message passing
            for layer in range(2):
                sh, tb = shard[layer], table[layer]
                if not skip_ag:
                    nc.gpsimd.collective_compute(
                        "AllGather", ALU.bypass,
                        replica_groups=[list(range(8))],
                        ins=[sh.ap()], outs=[tb.ap()])

                for b in range(NBLK):
                    gt = gp.tile([128, 4, GE * 2, REC], BF, tag="gt")
                    for q in range(4):
                        nc.gpsimd.dma_gather(
                            gt[:, q], tb.ap()[QROWS * q:QROWS * (q + 1), :],
                            P["eidx"][:, (b * 4 + q) * 64:(b * 4 + q + 1) * 64],
                            GE * SEG, GE * SEG, REC, single_packet=False)
                    gt2 = gp.tile([128, GE * 8, 128], BF, tag="gt2")
                    nc.gpsimd.dma_gather(
                        gt2[:], sh.ap()[:, 256:REC],
                        P["eidx2"][:, b * 256:(b + 1) * 256],
                        GE * 1024, GE * 1024, 128, elem_step=REC,
                        single_packet=False)
                    srec4 = sb.tile([128, GE, PAY], BF, tag="srec4")
                    nc.sync.dma_start(
                        srec4[:],
                        sh.ap()[b * GE * 128:(b + 1) * GE * 128, 0:PAY]
                        .rearrange("(g p) c -> p g c", g=GE))

                    den4 = sb.tile([128, GE, 2], F32, tag="den4")
                    pos = []
                    for jl in range(GE):
                        k = b * GE + jl
                        # per-edge logits: chunk 0 = self, 1+jj*4+q = edges
                        lg9 = sb.tile([128, 9, 2], F32, tag="lg9")
                        nc.vector.tensor_tensor(
                            lg9[:, 0, :], aa[layer][:, 4 * k:4 * k + 2],
                            aa[layer][:, 4 * k + 2:4 * k + 4], ALU.add)
                        for jj in range(2):
                            nc.vector.tensor_tensor(
                                lg9[:, 1 + 4 * jj:5 + 4 * jj, :],
                                gt[:, :, 2 * jl + jj, 256:258],
                                gt2[:, 8 * jl + 4 * jj:8 * jl + 4 * jj + 4, 2:4],
                                ALU.add)
                        we9 = sb.tile([128, 9, 2], F32, tag="we9")
                        lgp = sb.tile([128, 9, 2], F32, tag="lgp")
                        lrelu_exp(we9[:], lg9[:], lgp[:])
                        # weighted rhs: [w0*h0' | w1*h1' | w0 w1] per chunk;
                        # the muls read their bf16 scales back out of the
                        # rhs tile's own 256:258 columns (written first).
                        rhs = sb.tile([128, 9, 258], BF, tag="rhs")
                        nc.vector.tensor_copy(rhs[:, :, 256:258], we9[:])
                        nc.scalar.activation(rhs[:, 0, 0:128], srec4[:, jl, 0:128],
                                             AF.Copy, scale=we9[:, 0, 0:1])
                        nc.scalar.activation(rhs[:, 0, 128:256], srec4[:, jl, 128:256],
                                             AF.Copy, scale=we9[:, 0, 1:2])
                        for jj in range(2):
                            sl = slice(1 + 4 * jj, 5 + 4 * jj)
                            nc.vector.tensor_tensor(
                                rhs[:, sl, 0:128], gt[:, :, 2 * jl + jj, 0:128],
                                rhs[:, sl, 256:257].to_broadcast([128, 4, 128]),
                                ALU.mult)
                            nc.vector.tensor_tensor(
                                rhs[:, sl, 128:256], gt[:, :, 2 * jl + jj, 128:256],
                                rhs[:, sl, 257:258].to_broadcast([128, 4, 128]),
                                ALU.mult)
                        # one-hot scatter chunks (dst row within superchunk)
                        oh = sb.tile([128, 8, 128], BF, tag="oh")
                        nc.vector.tensor_tensor(
                            oh[:],
                            P["dstloc"][:, 8 * k:8 * k + 8].to_broadcast([128, 8, 128]),
                            P["iotam"][:], ALU.is_equal)
                        po = ps_po.tile([128, 258], F32, tag="po")
                        nc.tensor.matmul(po[:], lhsT=P["ident"][:], rhs=rhs[:, 0, :],
                                         start=True, stop=False)
                        for jc in range(8):
                            jj, q = jc // 4, jc % 4
                            nc.tensor.matmul(po[:], lhsT=oh[:, 2 * q + jj, :],
                                             rhs=rhs[:, 1 + jc, :],
                                             start=False, stop=(jc == 7))
                        nc.scalar.activation(den4[:, jl, :], po[:, 256:258],
                                             AF.Copy, bias=1e-16)
                        pos.append(po)
                    rd4 = sb.tile([128, GE, 2], F32, tag="rd4")
                    nc.vector.reciprocal(rd4[:], den4[:])

                    if layer == 0:
                        rec4b = sb.tile([128, GE, PAY], BF, tag="rec4b")
                    else:
                        y4 = sb.tile([128, GE, 128], F32, tag="y4")
                    for jl in range(GE):
                        k = b * GE + jl
                        po = pos[jl]
                        o2 = sb.tile([128, 256], BF, tag="o2")
                        nc.scalar.activation(o2[:, 0:128], po[:, 0:128],
                                             AF.Relu, scale=rd4[:, jl, 0:1])
                        nc.scalar.activation(o2[:, 128:256], po[:, 128:256],
                                             AF.Relu, scale=rd4[:, jl, 1:2])
                        # transpose o2 halves for the next matmul
                        hT = sb.tile([128, 2, 128], BF, tag="hT")
                        for r in range(2):
                            pt = ps_t.tile([128, 128], BF, tag="pt")
                            nc.tensor.transpose(pt[:], o2[:, r * 128:(r + 1) * 128],
                                                P["ident"][:])
                            nc.scalar.activation(hT[:, r, :], pt[:], AF.Copy)
                        if layer == 0:
                            ph2 = ps_t.tile([128, PAY], F32, tag="ph")
                            for r in range(2):
                                nc.tensor.matmul(ph2[:], lhsT=hT[:, r, :],
                                                 rhs=w2s[:, r, :],
                                                 start=(r == 0), stop=(r == 1))
                            nc.vector.tensor_tensor(rec4b[:, jl, :], ph2[:],
                                                    P["b2ext"][:], ALU.add)
                            nc.scalar.activation(aa[1][:, 4 * k:4 * k + 4],
                                                 ph2[:, 256:260], AF.Copy)
                        else:
                            pm2 = ps_t.tile([128, 128], F32, tag="pm")
                            for r in range(2):
                                nc.tensor.matmul(pm2[:], lhsT=wp1s[:, r, :],
                                                 rhs=hT[:, r, :],
                                                 start=(r == 0), stop=(r == 1))
                            t2T = sb.tile([128, 128], BF, tag="t2T")
                            nc.vector.tensor_scalar_add(t2T[:], pm2[:],
                                                        P["bp1col"][:, 0:1])
                            pyy = ps_t.tile([128, 128], F32, tag="pm")
                            nc.tensor.matmul(pyy[0:2, :], lhsT=P["wp2"][:],
                                             rhs=t2T[:], start=True, stop=True)
                            # sigmoid(x) = 0.5*tanh(0.5x) + 0.5
                            nc.scalar.activation(y4[0:2, jl, :], pyy[0:2, :],
                                                 AF.Tanh, scale=0.5,
                                                 bias=P["bp2col"][0:2, 0:1])
                            nc.vector.tensor_scalar(
                                y4[0:2, jl, :], y4[0:2, jl, :], 0.5, 0.5,
                                ALU.mult, ALU.add)
                    if layer == 0:
                        nc.scalar.dma_start(
                            shard[1].ap()[b * GE * 128:(b + 1) * GE * 128, 0:PAY]
                            .rearrange("(g p) c -> p g c", g=GE),
                            rec4b[:])
                    else:
                        nc.scalar.dma_start(
                            y_d.ap()[:, b * GE * 128:(b + 1) * GE * 128]
                            .rearrange("t (g p) -> t g p", g=GE),
                            y4[0:2, :, :])
    nc.compile()
    return nc


_NC_CACHE = None


def kernel(**inputs):
    global _NC_CACHE
    from concourse.bass_utils import run_bass_kernel_spmd

    plan = build_plan(inputs["edge_index"])
    cores, shared = make_core_inputs(plan, inputs)

    if _NC_CACHE is None:
        _NC_CACHE = build_nc()
    nc = _NC_CACHE

    in_maps = []
    for c in range(8):
        m = dict(shared)
        m.update(cores[c])
        in_maps.append({k: np.ascontiguousarray(v) for k, v in m.items()})

    res = run_bass_kernel_spmd(nc, in_maps, core_ids=list(range(8)))

    owner, sc_of, row_of = plan["owner"], plan["sc_of"], plan["row_of"]
    y = np.zeros((N, 2), dtype=np.float32)
    for c in range(8):
        yc = res.results[c]["y"]
        nodes = np.where(owner == c)[0]
        y[nodes] = yc[:, sc_of[nodes] * 128 + row_of[nodes]].T
    return y
